# revision 45
# baseline (speedup 1.0000x reference)
"""Trainium2 Bass kernel for nn_ContextualCritic (4-layer strided conv + segment mean).

Self-contained: kernel(**inputs) -> np.ndarray [2B, 8192].

Design v4 (per core, data-parallel over 8 cores, 512 images each):
 - All matmul operands bf16, fp32 PSUM accumulation, bf16 evictions.
 - Single fused pipeline: L1 -> L2 -> L3 stay SBUF-resident; L4 runs per
   32-image group. All weights preloaded to SBUF.
 - Parity-split activation layouts (stride-2 taps become dense plane reads).
 - L2 K=128 tap packing: l2i is img-major flat [128, 8, 1297]; lower 64
   partitions hold the 4 parity planes (18x18 with halo), upper 64 hold a
   flat 1-element-shifted copy made by one SBUF->SBUF DMA per image (every
   wrapped element lands on a halo zero / the pad element).  Reading the
   upper half at tap (kh,kw) yields tap (kh,kw+2), so taps pair into
   K=128 matmuls: 10 pairs + 5 singles (zeroed upper weights) = 15 MM
   slots per 2 images instead of 25 K=64 slots.
 - L1 produces M=64 outputs (no duplication); eviction ACT/DVE writes the
   lower 64 partitions only.
 - L2 evictions read PSUM directly (no DVE bank merge) with 2 ACTs per
   psum block via [pc, r, c2, i] access patterns.
 - Segment mean on host from [N,8192] features (sorted segment ids).
"""
import numpy as np

BLK = 8        # images per block
GRP = 32       # images per L4 group (nimg = ngrp*GRP)
NCORES = 8
L2IMG = 2 * 2 * 18 * 18      # 1296 elems per image in l2i
L2STR = L2IMG + 1            # +1 pad elem so the flat shift stays in-tile

_CACHE = {}

# L2 tap pairing: pairs (kh, kw) + (kh, kw+2), singles (kh, 4)
L2_PAIRS = [(kh, kw) for kh in range(5) for kw in range(2)]
L2_SINGLES = [(kh, 4) for kh in range(5)]


def _build_program(nimg, zero_bias):
    from concourse import bacc, mybir
    import concourse.tile as tile

    BF16 = mybir.dt.bfloat16
    F32 = mybir.dt.float32
    LRELU = mybir.ActivationFunctionType.Prelu
    MAX = mybir.AluOpType.max

    nblk = nimg // BLK

    nc = bacc.Bacc(None, target_bir_lowering=False)

    icd = nc.dram_tensor("ic", [75, nimg * 1024], BF16, kind="ExternalInput")
    w1d = nc.dram_tensor("w1", [128, 64], BF16, kind="ExternalInput")
    w2d = nc.dram_tensor("w2", [128, 15 * 128], BF16, kind="ExternalInput")
    w3d = nc.dram_tensor("w3", [128, 2 * 25 * 128], BF16, kind="ExternalInput")
    w4d = nc.dram_tensor("w4", [128, 2 * 25 * 512], BF16, kind="ExternalInput")
    b1d = nc.dram_tensor("b1", [128, 1], F32, kind="ExternalInput")
    b2d = nc.dram_tensor("b2", [128, 1], F32, kind="ExternalInput")
    b3d = nc.dram_tensor("b3", [128, 2], F32, kind="ExternalInput")
    b4d = nc.dram_tensor("b4", [128, 4], F32, kind="ExternalInput")
    fd = nc.dram_tensor("f", [128, 4, nimg, 16], F32, kind="ExternalOutput")

    with tile.TileContext(nc) as tc:
        with tc.tile_pool(name="const", bufs=1) as cst, \
             tc.tile_pool(name="work", bufs=1) as wk, \
             tc.tile_pool(name="ps", bufs=2, space="PSUM") as ps2, \
             tc.tile_pool(name="psl1", bufs=1, space="PSUM") as psl1p, \
             tc.tile_pool(name="stg", bufs=2) as stg:
            w1t = cst.tile([128, 64], BF16)
            nc.sync.dma_start(w1t[:], w1d[:, :])
            w2t = cst.tile([128, 15 * 128], BF16)
            w3t = cst.tile([128, 2 * 25 * 128], BF16)
            w4t = cst.tile([128, 2 * 25 * 512], BF16)
            b1t = cst.tile([128, 1], F32)
            nc.sync.dma_start(b1t[:], b1d[:, :])
            b2t = cst.tile([128, 1], F32)
            nc.sync.dma_start(b2t[:], b2d[:, :])
            b3t = cst.tile([128, 2], F32)
            nc.sync.dma_start(b3t[:], b3d[:, :])
            b4t = cst.tile([128, 4], F32)
            nc.sync.dma_start(b4t[:], b4d[:, :])
            a2t = cst.tile([128, 1], F32)
            nc.vector.memset(a2t[:], 0.2)

            icT = [wk.tile([128, BLK * 1024], BF16, name=f"ic{i}")
                   for i in range(3)]
            l2iT = [wk.tile([128, BLK, L2STR], BF16, name=f"l2i{i}")
                    for i in range(2)]
            l3iT = [wk.tile([128, 2, 10, 10, 2, BLK], BF16, name=f"l3i{i}")
                    for i in range(2)]
            l4iT = [wk.tile([128, 2, 6, 6, 2, GRP], BF16, name=f"l4i{i}")
                    for i in range(2)]
            for i in range(3):
                nc.vector.memset(icT[i][64:128, :], 0.0)
            for i in range(2):
                nc.vector.memset(l2iT[i][:], 0.0)
                nc.vector.memset(l3iT[i][:], 0.0)
                nc.vector.memset(l4iT[i][:], 0.0)

            # persistent L1 psum: 4 banks x 2 col-halves = 8 quadrants.
            # L4's two accumulators alias banks 2..3 (temporally disjoint:
            # the L1 blocks that touch banks 2..3 run ~40 MM slots after
            # the L4 fo evictions of the previous group have drained).
            psL1 = psl1p.tile([128, 4, 512], F32, name="psl1")

            def emit_l1_group(b, g):
                """L1 for images 2g, 2g+1 of blk b (2 psum halves each)."""
                ic = icT[b % 3]
                l2i = l2iT[b % 2]
                for img in (2 * g, 2 * g + 1):
                    cg, bp = img % 2, (img % 4) // 2
                    for h in range(2):
                        psb = 2 * img + h
                        nc.tensor.matmul(psL1[64 * cg:64 * cg + 64,
                                              2 * bp + h, :],
                                         w1t[:, :],
                                         ic[:, psb * 512:(psb + 1) * 512],
                                         start=True, stop=True)
                    # one 1024-elem eviction per image across the bank pair
                    # (psum col order [r2, a, b, c] makes (h r2) merge)
                    src = psL1[64 * cg:64 * cg + 64,
                               2 * bp:2 * bp + 2, 0:512].rearrange(
                        "p h (r a b c) -> p (a b) (h r) c", r=8, a=2, b=2)
                    l2f = l2i[0:64, img, 0:L2IMG].rearrange(
                        "p (a b r c) -> p a b r c", a=2, b=2, r=18)
                    dst = l2f[:, :, :, 1:17, 1:17].rearrange(
                        "p a b r c -> p (a b) r c")
                    if zero_bias and img % 4 == 1:
                        # DVE 2-op LeakyReLU eviction (bias known zero)
                        tmp = stg.tile([64, 1024], F32, tag="l1tmp")
                        tv = tmp[:, :].rearrange(
                            "p (ab r c) -> p ab r c", ab=4, r=16)
                        nc.vector.tensor_scalar_mul(tv, src, 0.2)
                        nc.vector.tensor_tensor(dst, src, tv, op=MAX)
                    else:
                        nc.scalar.activation(dst, src, LRELU,
                                             bias=b1t[0:64, :],
                                             alpha=a2t[0:64, :])
                    # shifted upper copy: one flat SBUF->SBUF DMA (HWDGE)
                    nc.sync.dma_start(l2i[64:128, img, 0:L2IMG],
                                      l2i[0:64, img, 1:L2STR])

            def emit_l2_psb(b, psb):
                """L2 psum block psb (images 2psb, 2psb+1) of blk b."""
                l2i = l2iT[b % 2]
                l3i = l3iT[b % 2]
                j0 = 2 * psb
                mv = l2i[:, j0:j0 + 2, 0:L2IMG].rearrange(
                    "p i (a b r c) -> p i a b r c", a=2, b=2, r=18)
                ps = ps2.tile([128, 2, 16, 16], F32, tag="l2ps")
                for idx, (kh, kw) in enumerate(L2_PAIRS):
                    nc.tensor.matmul(
                        ps[:], w2t[:, idx * 128:(idx + 1) * 128],
                        mv[:, :, kh % 2, kw % 2, kh // 2:kh // 2 + 16,
                           kw // 2:kw // 2 + 16],
                        start=(idx == 0), stop=False)
                for s, (kh, kw) in enumerate(L2_SINGLES):
                    idx = 10 + s
                    nc.tensor.matmul(
                        ps[:], w2t[:, idx * 128:(idx + 1) * 128],
                        mv[:, :, kh % 2, 0, kh // 2:kh // 2 + 16, 2:18],
                        start=False, stop=(s == 4))
                # evict into l3i (pc-inner layout): 2 merged 256-elem ops
                for pr in range(2):
                    src = ps[:, :, pr::2, :]
                    dst = l3i[:, pr, 1:9, 1:9, 0:2, j0:j0 + 2].rearrange(
                        "p r c pc i -> p i r (c pc)")
                    if zero_bias and pr == 1:
                        tmp = stg.tile([128, 256], F32, tag="l2tmp")
                        tv = tmp[:, 0:256].rearrange(
                            "p (i r c) -> p i r c", i=2, r=8)
                        nc.vector.tensor_scalar_mul(tv, src, 0.2)
                        nc.vector.tensor_tensor(dst, src, tv, op=MAX)
                    else:
                        nc.scalar.activation(dst, src, LRELU,
                                             bias=b2t[:, :], alpha=a2t[:, :])

            def emit_l3(b):
                """L3 for blk b: 2 ci planes x 25 taps, psum [r, c, img]."""
                l3i = l3iT[b % 2]
                sb4 = b % 4
                for cp in range(2):
                    ps3 = ps2.tile([128, 8, 8, BLK], F32, tag="l3ps")
                    for tap in range(25):
                        kh, kw = tap // 5, tap % 5
                        nc.tensor.matmul(
                            ps3[:],
                            w3t[:, (cp * 25 + tap) * 128:
                                (cp * 25 + tap + 1) * 128],
                            l3i[:, kh % 2, kh // 2:kh // 2 + 8,
                                kw // 2:kw // 2 + 8, kw % 2, :],
                            start=(tap == 0), stop=(tap == 24))
                    for pr in range(2):
                        nc.scalar.activation(
                            l4iT[cp][:, pr, 1:5, 1:5, 0:2,
                                     sb4 * BLK:(sb4 + 1) * BLK]
                            .rearrange("p r c pc i -> p r (c pc) i"),
                            ps3[:, pr::2, :, :],
                            LRELU, bias=b3t[:, cp:cp + 1], alpha=a2t[:, :])

            def emit_l4(grp):
                """L4 over a completed 32-image group: 4 sequential q-passes
                (one 50-MM chain each) so no fo eviction head-blocks the ACT
                queue for more than one chain."""
                for q in range(4):
                    p4 = psL1[:, 2 + q % 2, :].rearrange(
                        "p (r c i) -> p r c i", r=4, c=4)
                    for i4 in range(50):
                        cip, tap = i4 // 25, i4 % 25
                        kh, kw = tap // 5, tap % 5
                        w0 = (cip * 25 + tap) * 512 + q * 128
                        nc.tensor.matmul(
                            p4,
                            w4t[:, w0:w0 + 128],
                            l4iT[cip][:, kh % 2,
                                      kh // 2:kh // 2 + 4,
                                      kw // 2:kw // 2 + 4, kw % 2, :],
                            start=(i4 == 0), stop=(i4 == 49))
                    fo = stg.tile([128, GRP, 16], F32, tag="fo")
                    nc.scalar.activation(
                        fo[:], p4.rearrange("p r c i -> p i (r c)"),
                        LRELU, bias=b4t[:, q:q + 1], alpha=a2t[:, :])
                    nc.scalar.dma_start(
                        fd[:, q, grp * GRP:(grp + 1) * GRP, :],
                        fo[:])

            def dma_ic(b):
                c0 = b * BLK * 1024
                ic = icT[b % 3]
                nc.sync.dma_start(ic[0:38, :], icd[0:38, c0:c0 + BLK * 1024])
                nc.sync.dma_start(ic[38:75, :], icd[38:75, c0:c0 + BLK * 1024])

            # software pipeline: [L1(b) interleaved with L2(b-1)], L3(b-2), L4
            # ic first so L1(0) starts immediately; big weights follow
            # (w2/w3/w4 are first needed 1/2/5 iterations in)
            dma_ic(0)
            dma_ic(1)
            nc.scalar.dma_start(w2t[:], w2d[:, :])
            nc.scalar.dma_start(w3t[:], w3d[:, :])
            nc.scalar.dma_start(w4t[:], w4d[:, :])
            for b in range(nblk + 2):
                if b < nblk:
                    for g in range(4):
                        if b >= 1:
                            emit_l2_psb(b - 1, g)
                        emit_l1_group(b, g)
                    if b + 2 < nblk:
                        dma_ic(b + 2)
                elif b == nblk:
                    for g in range(4):
                        emit_l2_psb(b - 1, g)
                if 2 <= b <= nblk + 1:
                    emit_l3(b - 2)
                    if (b - 2) % 4 == 3:
                        emit_l4((b - 2) // 4)
    nc.compile()
    return nc


def _prep_inputs(x, W1, b1, W2, b2, W3, b3, W4, b4, nimg):
    """Host preprocessing -> per-core in_maps (shared weight arrays)."""
    import ml_dtypes
    bf16 = ml_dtypes.bfloat16
    f32 = np.float32
    n = x.shape[0]
    ncores = n // nimg
    xpad = np.pad(np.asarray(x, dtype=f32), ((0, 0), (0, 0), (2, 2), (2, 2)))
    s = xpad.strides
    v = np.lib.stride_tricks.as_strided(
        xpad, shape=(n, 3, 5, 5, 32, 32),
        strides=(s[0], s[1], s[2], s[3], 2 * s[2], 2 * s[3]))
    # column order per image: [h(2), r2(8), pr(2), pc(2), c2(16)]
    # row 32 = h*16 + r2*2 + pr ; col 32 = c2*2 + pc
    vr = v.reshape(n, 3, 5, 5, 2, 8, 2, 16, 2)      # rows->(h,r2,pr) cols->(c2,pc)
    vp = vr.transpose(1, 2, 3, 0, 4, 5, 6, 8, 7)    # [3,5,5,n,h,r2,pr,pc,c2]
    ic_all = np.ascontiguousarray(
        vp.reshape(75, n, 1024).astype(bf16))

    w1l = np.ascontiguousarray(
        np.asarray(W1, f32).transpose(1, 2, 3, 0).reshape(75, 64))
    w1h = np.zeros((128, 64), f32)
    w1h[0:75, 0:64] = w1l
    b1h = np.zeros((128, 1), f32)
    b1h[0:64, 0] = np.asarray(b1, f32)

    w2h = np.zeros((128, 15 * 128), f32)
    W2f = np.asarray(W2, f32)
    for idx, (kh, kw) in enumerate(L2_PAIRS):
        w2h[0:64, idx * 128:(idx + 1) * 128] = W2f[:, :, kh, kw].T
        w2h[64:128, idx * 128:(idx + 1) * 128] = W2f[:, :, kh, kw + 2].T
    for s_, (kh, kw) in enumerate(L2_SINGLES):
        idx = 10 + s_
        w2h[0:64, idx * 128:(idx + 1) * 128] = W2f[:, :, kh, kw].T
    b2h = np.asarray(b2, f32).reshape(128, 1)

    w3h = np.zeros((128, 2 * 25 * 128), f32)
    for cp in range(2):
        for t in range(25):
            kh, kw = t // 5, t % 5
            w3h[:, (cp * 25 + t) * 128:(cp * 25 + t + 1) * 128] = \
                np.asarray(W3, f32)[cp * 128:(cp + 1) * 128, :, kh, kw].T
    b3h = np.ascontiguousarray(
        np.asarray(b3, f32).reshape(2, 128).T)                   # [128,2]

    # w4 SBUF-resident layout: [(cip*25+tap)*512 + q*128 + m] columns
    w4h = np.zeros((128, 2 * 25 * 512), f32)
    for cip in range(2):
        for t in range(25):
            kh, kw = t // 5, t % 5
            w4h[:, (cip * 25 + t) * 512:(cip * 25 + t + 1) * 512] = \
                np.asarray(W4, f32)[:, cip * 128:(cip + 1) * 128, kh, kw].T
    b4h = np.ascontiguousarray(
        np.asarray(b4, f32).reshape(4, 128).T)                   # [128,4]

    w1h = w1h.astype(bf16)
    w2h = w2h.astype(bf16)
    w3h = w3h.astype(bf16)
    w4h = w4h.astype(bf16)

    in_maps = []
    for c in range(ncores):
        ic = np.ascontiguousarray(
            ic_all[:, c * nimg:(c + 1) * nimg, :].reshape(75, nimg * 1024))
        in_maps.append({"ic": ic, "w1": w1h, "w2": w2h, "w3": w3h,
                        "w4": w4h, "b1": b1h, "b2": b2h, "b3": b3h,
                        "b4": b4h})
    return in_maps


def _run(inputs, trace=False, nimg=512, ncores=NCORES):
    from concourse.bass_utils import run_bass_kernel_spmd

    zero_bias = not np.any(np.asarray(inputs["b1"]))
    key = (nimg, ncores, zero_bias)
    if key not in _CACHE:
        _CACHE[key] = _build_program(nimg, zero_bias)
    nc = _CACHE[key]

    in_maps = _prep_inputs(
        inputs["x"], inputs["W1"], inputs["b1"], inputs["W2"], inputs["b2"],
        inputs["W3"], inputs["b3"], inputs["W4"], inputs["b4"], nimg)

    res = run_bass_kernel_spmd(nc, in_maps, core_ids=list(range(ncores)),
                               trace=trace)
    feats = np.concatenate(
        [r["f"].transpose(2, 1, 0, 3).reshape(nimg, 8192)
         for r in res.results], axis=0)                          # [N, 8192]
    return feats, res


def kernel(**inputs):
    x = np.asarray(inputs["x"])
    n = x.shape[0]
    nimg = n // NCORES
    feats, _ = _run(inputs, trace=False, nimg=nimg)

    if int(np.asarray(inputs.get("is_local", 1))) == 0:
        return feats.astype(np.float32)

    batch_size = int(np.asarray(inputs["batch_size"]))
    seg = np.asarray(inputs["f_obj_to_img"]).astype(np.int64)
    nh = n // 2
    fake, real = feats[:nh], feats[nh:]
    counts = np.bincount(seg, minlength=batch_size).astype(np.float32)
    denom = np.maximum(counts, 1.0)[:, None]
    fsum = np.zeros((batch_size, 8192), np.float32)
    rsum = np.zeros((batch_size, 8192), np.float32)
    np.add.at(fsum, seg, fake)
    np.add.at(rsum, seg, real)
    favg = np.where((counts > 0)[:, None], fsum / denom, 0.0)
    ravg = np.where((counts > 0)[:, None], rsum / denom, 0.0)
    return np.concatenate([favg, ravg], axis=0).astype(np.float32)


# revision 46
# speedup vs baseline: 1.0116x; 1.0116x over previous
"""Trainium2 Bass kernel for nn_ContextualCritic (4-layer strided conv + segment mean).

Self-contained: kernel(**inputs) -> np.ndarray [2B, 8192].

Design v4 (per core, data-parallel over 8 cores, 512 images each):
 - All matmul operands bf16, fp32 PSUM accumulation, bf16 evictions.
 - Single fused pipeline: L1 -> L2 -> L3 stay SBUF-resident; L4 runs per
   32-image group. All weights preloaded to SBUF.
 - Parity-split activation layouts (stride-2 taps become dense plane reads).
 - L2 K=128 tap packing: l2i is img-major flat [128, 8, 1297]; lower 64
   partitions hold the 4 parity planes (18x18 with halo), upper 64 hold a
   flat 1-element-shifted copy made by one SBUF->SBUF DMA per image (every
   wrapped element lands on a halo zero / the pad element).  Reading the
   upper half at tap (kh,kw) yields tap (kh,kw+2), so taps pair into
   K=128 matmuls: 10 pairs + 5 singles (zeroed upper weights) = 15 MM
   slots per 2 images instead of 25 K=64 slots.
 - L1 produces M=64 outputs (no duplication); eviction ACT/DVE writes the
   lower 64 partitions only.
 - L2 evictions read PSUM directly (no DVE bank merge) with 2 ACTs per
   psum block via [pc, r, c2, i] access patterns.
 - Segment mean on host from [N,8192] features (sorted segment ids).
"""
import numpy as np

BLK = 8        # images per block
GRP = 32       # images per L4 group (nimg = ngrp*GRP)
NCORES = 8
L2IMG = 2 * 2 * 18 * 18      # 1296 elems per image in l2i
L2STR = L2IMG + 1            # +1 pad elem so the flat shift stays in-tile

_CACHE = {}

# L2 tap pairing: pairs (kh, kw) + (kh, kw+2), singles (kh, 4)
L2_PAIRS = [(kh, kw) for kh in range(5) for kw in range(2)]
L2_SINGLES = [(kh, 4) for kh in range(5)]


def _build_program(nimg, zero_bias):
    from concourse import bacc, mybir
    import concourse.tile as tile

    BF16 = mybir.dt.bfloat16
    F32 = mybir.dt.float32
    LRELU = mybir.ActivationFunctionType.Prelu
    MAX = mybir.AluOpType.max

    nblk = nimg // BLK

    nc = bacc.Bacc(None, target_bir_lowering=False)

    icd = nc.dram_tensor("ic", [75, nimg * 1024], BF16, kind="ExternalInput")
    w1d = nc.dram_tensor("w1", [128, 64], BF16, kind="ExternalInput")
    w2d = nc.dram_tensor("w2", [128, 15 * 128], BF16, kind="ExternalInput")
    w3d = nc.dram_tensor("w3", [128, 2 * 25 * 128], BF16, kind="ExternalInput")
    w4d = nc.dram_tensor("w4", [128, 2 * 25 * 512], BF16, kind="ExternalInput")
    b1d = nc.dram_tensor("b1", [128, 1], F32, kind="ExternalInput")
    b2d = nc.dram_tensor("b2", [128, 1], F32, kind="ExternalInput")
    b3d = nc.dram_tensor("b3", [128, 2], F32, kind="ExternalInput")
    b4d = nc.dram_tensor("b4", [128, 4], F32, kind="ExternalInput")
    fd = nc.dram_tensor("f", [128, 4, nimg, 16], F32, kind="ExternalOutput")

    with tile.TileContext(nc) as tc:
        with tc.tile_pool(name="const", bufs=1) as cst, \
             tc.tile_pool(name="work", bufs=1) as wk, \
             tc.tile_pool(name="ps", bufs=2, space="PSUM") as ps2, \
             tc.tile_pool(name="psl1", bufs=1, space="PSUM") as psl1p, \
             tc.tile_pool(name="stg", bufs=2) as stg:
            w1t = cst.tile([128, 64], BF16)
            nc.sync.dma_start(w1t[:], w1d[:, :])
            w2t = cst.tile([128, 15 * 128], BF16)
            w3t = cst.tile([128, 2 * 25 * 128], BF16)
            w4t = cst.tile([128, 2 * 25 * 512], BF16)
            b1t = cst.tile([128, 1], F32)
            nc.sync.dma_start(b1t[:], b1d[:, :])
            b2t = cst.tile([128, 1], F32)
            nc.sync.dma_start(b2t[:], b2d[:, :])
            b3t = cst.tile([128, 2], F32)
            nc.sync.dma_start(b3t[:], b3d[:, :])
            b4t = cst.tile([128, 4], F32)
            nc.sync.dma_start(b4t[:], b4d[:, :])
            a2t = cst.tile([128, 1], F32)
            nc.vector.memset(a2t[:], 0.2)

            icT = [wk.tile([128, BLK * 1024], BF16, name=f"ic{i}")
                   for i in range(2)]
            l2iT = [wk.tile([128, BLK, L2STR], BF16, name=f"l2i{i}")
                    for i in range(2)]
            l3iT = [wk.tile([128, 2, 10, 10, 2, BLK], BF16, name=f"l3i{i}")
                    for i in range(2)]
            l4iT = [wk.tile([128, 2, 6, 6, 2, GRP], BF16, name=f"l4i{i}")
                    for i in range(2)]
            for i in range(2):
                nc.vector.memset(icT[i][64:128, :], 0.0)
                nc.vector.memset(l2iT[i][:], 0.0)
                nc.vector.memset(l3iT[i][:], 0.0)
                nc.vector.memset(l4iT[i][:], 0.0)

            # persistent L1 psum: 4 banks x 2 col-halves = 8 quadrants.
            # L4's two accumulators alias banks 2..3 (temporally disjoint:
            # the L1 blocks that touch banks 2..3 run ~40 MM slots after
            # the L4 fo evictions of the previous group have drained).
            psL1 = psl1p.tile([128, 4, 512], F32, name="psl1")

            def emit_l1_group(b, g):
                """L1 for images 2g, 2g+1 of blk b (2 psum halves each)."""
                ic = icT[b % 2]
                l2i = l2iT[b % 2]
                for img in (2 * g, 2 * g + 1):
                    cg, bp = img % 2, (img % 4) // 2
                    for h in range(2):
                        psb = 2 * img + h
                        nc.tensor.matmul(psL1[64 * cg:64 * cg + 64,
                                              2 * bp + h, :],
                                         w1t[:, :],
                                         ic[:, psb * 512:(psb + 1) * 512],
                                         start=True, stop=True)
                    # one 1024-elem eviction per image across the bank pair
                    # (psum col order [r2, a, b, c] makes (h r2) merge)
                    src = psL1[64 * cg:64 * cg + 64,
                               2 * bp:2 * bp + 2, 0:512].rearrange(
                        "p h (r a b c) -> p (a b) (h r) c", r=8, a=2, b=2)
                    l2f = l2i[0:64, img, 0:L2IMG].rearrange(
                        "p (a b r c) -> p a b r c", a=2, b=2, r=18)
                    dst = l2f[:, :, :, 1:17, 1:17].rearrange(
                        "p a b r c -> p (a b) r c")
                    if zero_bias and img % 4 == 1:
                        # DVE 2-op LeakyReLU eviction (bias known zero)
                        tmp = stg.tile([64, 1024], F32, tag="l1tmp")
                        tv = tmp[:, :].rearrange(
                            "p (ab r c) -> p ab r c", ab=4, r=16)
                        nc.vector.tensor_scalar_mul(tv, src, 0.2)
                        nc.vector.tensor_tensor(dst, src, tv, op=MAX)
                    else:
                        nc.scalar.activation(dst, src, LRELU,
                                             bias=b1t[0:64, :],
                                             alpha=a2t[0:64, :])
                    # shifted upper copy: one flat SBUF->SBUF DMA (HWDGE)
                    nc.sync.dma_start(l2i[64:128, img, 0:L2IMG],
                                      l2i[0:64, img, 1:L2STR])

            def emit_l2_psb(b, psb):
                """L2 psum block psb (images 2psb, 2psb+1) of blk b."""
                l2i = l2iT[b % 2]
                l3i = l3iT[b % 2]
                j0 = 2 * psb
                mv = l2i[:, j0:j0 + 2, 0:L2IMG].rearrange(
                    "p i (a b r c) -> p i a b r c", a=2, b=2, r=18)
                ps = ps2.tile([128, 2, 16, 16], F32, tag="l2ps")
                for idx, (kh, kw) in enumerate(L2_PAIRS):
                    nc.tensor.matmul(
                        ps[:], w2t[:, idx * 128:(idx + 1) * 128],
                        mv[:, :, kh % 2, kw % 2, kh // 2:kh // 2 + 16,
                           kw // 2:kw // 2 + 16],
                        start=(idx == 0), stop=False)
                for s, (kh, kw) in enumerate(L2_SINGLES):
                    idx = 10 + s
                    nc.tensor.matmul(
                        ps[:], w2t[:, idx * 128:(idx + 1) * 128],
                        mv[:, :, kh % 2, 0, kh // 2:kh // 2 + 16, 2:18],
                        start=False, stop=(s == 4))
                # evict into l3i (pc-inner layout): 2 merged 256-elem ops
                for pr in range(2):
                    src = ps[:, :, pr::2, :]
                    dst = l3i[:, pr, 1:9, 1:9, 0:2, j0:j0 + 2].rearrange(
                        "p r c pc i -> p i r (c pc)")
                    if zero_bias and pr == 1:
                        tmp = stg.tile([128, 256], F32, tag="l2tmp")
                        tv = tmp[:, 0:256].rearrange(
                            "p (i r c) -> p i r c", i=2, r=8)
                        nc.vector.tensor_scalar_mul(tv, src, 0.2)
                        nc.vector.tensor_tensor(dst, src, tv, op=MAX)
                    else:
                        nc.scalar.activation(dst, src, LRELU,
                                             bias=b2t[:, :], alpha=a2t[:, :])

            def emit_l3(b):
                """L3 for blk b: 2 ci planes x 25 taps, psum [r, c, img]."""
                l3i = l3iT[b % 2]
                sb4 = b % 4
                for cp in range(2):
                    ps3 = ps2.tile([128, 8, 8, BLK], F32, tag="l3ps")
                    for tap in range(25):
                        kh, kw = tap // 5, tap % 5
                        nc.tensor.matmul(
                            ps3[:],
                            w3t[:, (cp * 25 + tap) * 128:
                                (cp * 25 + tap + 1) * 128],
                            l3i[:, kh % 2, kh // 2:kh // 2 + 8,
                                kw // 2:kw // 2 + 8, kw % 2, :],
                            start=(tap == 0), stop=(tap == 24))
                    for pr in range(2):
                        nc.scalar.activation(
                            l4iT[cp][:, pr, 1:5, 1:5, 0:2,
                                     sb4 * BLK:(sb4 + 1) * BLK]
                            .rearrange("p r c pc i -> p r (c pc) i"),
                            ps3[:, pr::2, :, :],
                            LRELU, bias=b3t[:, cp:cp + 1], alpha=a2t[:, :])

            def emit_l4(grp):
                """L4 over a completed 32-image group: 4 sequential q-passes
                (one 50-MM chain each) so no fo eviction head-blocks the ACT
                queue for more than one chain."""
                for q in range(4):
                    p4 = psL1[:, 2 + q % 2, :].rearrange(
                        "p (r c i) -> p r c i", r=4, c=4)
                    for i4 in range(50):
                        cip, tap = i4 // 25, i4 % 25
                        kh, kw = tap // 5, tap % 5
                        w0 = (cip * 25 + tap) * 512 + q * 128
                        nc.tensor.matmul(
                            p4,
                            w4t[:, w0:w0 + 128],
                            l4iT[cip][:, kh % 2,
                                      kh // 2:kh // 2 + 4,
                                      kw // 2:kw // 2 + 4, kw % 2, :],
                            start=(i4 == 0), stop=(i4 == 49))
                    fo = stg.tile([128, GRP, 16], F32, tag="fo")
                    nc.scalar.activation(
                        fo[:], p4.rearrange("p r c i -> p i (r c)"),
                        LRELU, bias=b4t[:, q:q + 1], alpha=a2t[:, :])
                    nc.scalar.dma_start(
                        fd[:, q, grp * GRP:(grp + 1) * GRP, :],
                        fo[:])

            def dma_ic(b):
                c0 = b * BLK * 1024
                ic = icT[b % 2]
                nc.sync.dma_start(ic[0:38, :], icd[0:38, c0:c0 + BLK * 1024])
                nc.sync.dma_start(ic[38:75, :], icd[38:75, c0:c0 + BLK * 1024])

            # software pipeline: [L1(b) interleaved with L2(b-1)], L3(b-2), L4
            # ic first so L1(0) starts immediately; big weights follow
            # (w2/w3/w4 are first needed 1/2/5 iterations in)
            dma_ic(0)
            dma_ic(1)
            nc.scalar.dma_start(w2t[:], w2d[:, :])
            nc.scalar.dma_start(w3t[:], w3d[:, :])
            nc.scalar.dma_start(w4t[:], w4d[:, :])
            for b in range(nblk + 2):
                if b < nblk:
                    for g in range(4):
                        if b >= 1:
                            emit_l2_psb(b - 1, g)
                        emit_l1_group(b, g)
                    if b + 2 < nblk:
                        dma_ic(b + 2)
                elif b == nblk:
                    for g in range(4):
                        emit_l2_psb(b - 1, g)
                if 2 <= b <= nblk + 1:
                    emit_l3(b - 2)
                    if (b - 2) % 4 == 3:
                        emit_l4((b - 2) // 4)
    nc.compile()
    return nc


def _prep_inputs(x, W1, b1, W2, b2, W3, b3, W4, b4, nimg):
    """Host preprocessing -> per-core in_maps (shared weight arrays)."""
    import ml_dtypes
    bf16 = ml_dtypes.bfloat16
    f32 = np.float32
    n = x.shape[0]
    ncores = n // nimg
    xpad = np.pad(np.asarray(x, dtype=f32), ((0, 0), (0, 0), (2, 2), (2, 2)))
    s = xpad.strides
    v = np.lib.stride_tricks.as_strided(
        xpad, shape=(n, 3, 5, 5, 32, 32),
        strides=(s[0], s[1], s[2], s[3], 2 * s[2], 2 * s[3]))
    # column order per image: [h(2), r2(8), pr(2), pc(2), c2(16)]
    # row 32 = h*16 + r2*2 + pr ; col 32 = c2*2 + pc
    vr = v.reshape(n, 3, 5, 5, 2, 8, 2, 16, 2)      # rows->(h,r2,pr) cols->(c2,pc)
    vp = vr.transpose(1, 2, 3, 0, 4, 5, 6, 8, 7)    # [3,5,5,n,h,r2,pr,pc,c2]
    ic_all = np.ascontiguousarray(
        vp.reshape(75, n, 1024).astype(bf16))

    w1l = np.ascontiguousarray(
        np.asarray(W1, f32).transpose(1, 2, 3, 0).reshape(75, 64))
    w1h = np.zeros((128, 64), f32)
    w1h[0:75, 0:64] = w1l
    b1h = np.zeros((128, 1), f32)
    b1h[0:64, 0] = np.asarray(b1, f32)

    w2h = np.zeros((128, 15 * 128), f32)
    W2f = np.asarray(W2, f32)
    for idx, (kh, kw) in enumerate(L2_PAIRS):
        w2h[0:64, idx * 128:(idx + 1) * 128] = W2f[:, :, kh, kw].T
        w2h[64:128, idx * 128:(idx + 1) * 128] = W2f[:, :, kh, kw + 2].T
    for s_, (kh, kw) in enumerate(L2_SINGLES):
        idx = 10 + s_
        w2h[0:64, idx * 128:(idx + 1) * 128] = W2f[:, :, kh, kw].T
    b2h = np.asarray(b2, f32).reshape(128, 1)

    w3h = np.zeros((128, 2 * 25 * 128), f32)
    for cp in range(2):
        for t in range(25):
            kh, kw = t // 5, t % 5
            w3h[:, (cp * 25 + t) * 128:(cp * 25 + t + 1) * 128] = \
                np.asarray(W3, f32)[cp * 128:(cp + 1) * 128, :, kh, kw].T
    b3h = np.ascontiguousarray(
        np.asarray(b3, f32).reshape(2, 128).T)                   # [128,2]

    # w4 SBUF-resident layout: [(cip*25+tap)*512 + q*128 + m] columns
    w4h = np.zeros((128, 2 * 25 * 512), f32)
    for cip in range(2):
        for t in range(25):
            kh, kw = t // 5, t % 5
            w4h[:, (cip * 25 + t) * 512:(cip * 25 + t + 1) * 512] = \
                np.asarray(W4, f32)[:, cip * 128:(cip + 1) * 128, kh, kw].T
    b4h = np.ascontiguousarray(
        np.asarray(b4, f32).reshape(4, 128).T)                   # [128,4]

    w1h = w1h.astype(bf16)
    w2h = w2h.astype(bf16)
    w3h = w3h.astype(bf16)
    w4h = w4h.astype(bf16)

    in_maps = []
    for c in range(ncores):
        ic = np.ascontiguousarray(
            ic_all[:, c * nimg:(c + 1) * nimg, :].reshape(75, nimg * 1024))
        in_maps.append({"ic": ic, "w1": w1h, "w2": w2h, "w3": w3h,
                        "w4": w4h, "b1": b1h, "b2": b2h, "b3": b3h,
                        "b4": b4h})
    return in_maps


def _run(inputs, trace=False, nimg=512, ncores=NCORES):
    from concourse.bass_utils import run_bass_kernel_spmd

    zero_bias = not np.any(np.asarray(inputs["b1"]))
    key = (nimg, ncores, zero_bias)
    if key not in _CACHE:
        _CACHE[key] = _build_program(nimg, zero_bias)
    nc = _CACHE[key]

    in_maps = _prep_inputs(
        inputs["x"], inputs["W1"], inputs["b1"], inputs["W2"], inputs["b2"],
        inputs["W3"], inputs["b3"], inputs["W4"], inputs["b4"], nimg)

    res = run_bass_kernel_spmd(nc, in_maps, core_ids=list(range(ncores)),
                               trace=trace)
    feats = np.concatenate(
        [r["f"].transpose(2, 1, 0, 3).reshape(nimg, 8192)
         for r in res.results], axis=0)                          # [N, 8192]
    return feats, res


def kernel(**inputs):
    x = np.asarray(inputs["x"])
    n = x.shape[0]
    nimg = n // NCORES
    feats, _ = _run(inputs, trace=False, nimg=nimg)

    if int(np.asarray(inputs.get("is_local", 1))) == 0:
        return feats.astype(np.float32)

    batch_size = int(np.asarray(inputs["batch_size"]))
    seg = np.asarray(inputs["f_obj_to_img"]).astype(np.int64)
    nh = n // 2
    fake, real = feats[:nh], feats[nh:]
    counts = np.bincount(seg, minlength=batch_size).astype(np.float32)
    denom = np.maximum(counts, 1.0)[:, None]
    fsum = np.zeros((batch_size, 8192), np.float32)
    rsum = np.zeros((batch_size, 8192), np.float32)
    np.add.at(fsum, seg, fake)
    np.add.at(rsum, seg, real)
    favg = np.where((counts > 0)[:, None], fsum / denom, 0.0)
    ravg = np.where((counts > 0)[:, None], rsum / denom, 0.0)
    return np.concatenate([favg, ravg], axis=0).astype(np.float32)


# revision 47
# speedup vs baseline: 1.0140x; 1.0024x over previous
"""Trainium2 Bass kernel for nn_ContextualCritic (4-layer strided conv + segment mean).

Self-contained: kernel(**inputs) -> np.ndarray [2B, 8192].

Design v4 (per core, data-parallel over 8 cores, 512 images each):
 - All matmul operands bf16, fp32 PSUM accumulation, bf16 evictions.
 - Single fused pipeline: L1 -> L2 -> L3 stay SBUF-resident; L4 runs per
   32-image group. All weights preloaded to SBUF.
 - Parity-split activation layouts (stride-2 taps become dense plane reads).
 - L2 K=128 tap packing: l2i is img-major flat [128, 8, 1297]; lower 64
   partitions hold the 4 parity planes (18x18 with halo), upper 64 hold a
   flat 1-element-shifted copy made by one SBUF->SBUF DMA per image (every
   wrapped element lands on a halo zero / the pad element).  Reading the
   upper half at tap (kh,kw) yields tap (kh,kw+2), so taps pair into
   K=128 matmuls: 10 pairs + 5 singles (zeroed upper weights) = 15 MM
   slots per 2 images instead of 25 K=64 slots.
 - L1 produces M=64 outputs (no duplication); eviction ACT/DVE writes the
   lower 64 partitions only.
 - L2 evictions read PSUM directly (no DVE bank merge) with 2 ACTs per
   psum block via [pc, r, c2, i] access patterns.
 - Segment mean on host from [N,8192] features (sorted segment ids).
"""
import numpy as np

BLK = 8        # images per block
GRP = 32       # images per L4 group (nimg = ngrp*GRP)
NCORES = 8
L2IMG = 2 * 2 * 18 * 18      # 1296 elems per image in l2i
L2STR = L2IMG + 1            # +1 pad elem so the flat shift stays in-tile

_CACHE = {}

# L2 tap pairing: pairs (kh, kw) + (kh, kw+2), singles (kh, 4)
L2_PAIRS = [(kh, kw) for kh in range(5) for kw in range(2)]
L2_SINGLES = [(kh, 4) for kh in range(5)]


def _build_program(nimg, zero_bias):
    from concourse import bacc, mybir
    import concourse.tile as tile

    BF16 = mybir.dt.bfloat16
    F32 = mybir.dt.float32
    LRELU = mybir.ActivationFunctionType.Prelu
    MAX = mybir.AluOpType.max

    nblk = nimg // BLK

    nc = bacc.Bacc(None, target_bir_lowering=False)

    icd = nc.dram_tensor("ic", [75, nimg * 1024], BF16, kind="ExternalInput")
    w1d = nc.dram_tensor("w1", [128, 64], BF16, kind="ExternalInput")
    w2d = nc.dram_tensor("w2", [128, 15 * 128], BF16, kind="ExternalInput")
    w3d = nc.dram_tensor("w3", [128, 2 * 25 * 128], BF16, kind="ExternalInput")
    w4d = nc.dram_tensor("w4", [128, 2 * 25 * 512], BF16, kind="ExternalInput")
    b1d = nc.dram_tensor("b1", [128, 1], F32, kind="ExternalInput")
    b2d = nc.dram_tensor("b2", [128, 1], F32, kind="ExternalInput")
    b3d = nc.dram_tensor("b3", [128, 2], F32, kind="ExternalInput")
    b4d = nc.dram_tensor("b4", [128, 4], F32, kind="ExternalInput")
    fd = nc.dram_tensor("f", [128, 4, nimg, 16], F32, kind="ExternalOutput")

    with tile.TileContext(nc) as tc:
        with tc.tile_pool(name="const", bufs=1) as cst, \
             tc.tile_pool(name="work", bufs=1) as wk, \
             tc.tile_pool(name="ps", bufs=2, space="PSUM") as ps2, \
             tc.tile_pool(name="psl1", bufs=1, space="PSUM") as psl1p, \
             tc.tile_pool(name="stg", bufs=2) as stg:
            w1t = cst.tile([128, 64], BF16)
            nc.sync.dma_start(w1t[:], w1d[:, :])
            w2t = cst.tile([128, 15 * 128], BF16)
            w3t = cst.tile([128, 2 * 25 * 128], BF16)
            w4t = cst.tile([128, 2 * 25 * 512], BF16)
            b1t = cst.tile([128, 1], F32)
            nc.sync.dma_start(b1t[:], b1d[:, :])
            b2t = cst.tile([128, 1], F32)
            nc.sync.dma_start(b2t[:], b2d[:, :])
            b3t = cst.tile([128, 2], F32)
            nc.sync.dma_start(b3t[:], b3d[:, :])
            b4t = cst.tile([128, 4], F32)
            nc.sync.dma_start(b4t[:], b4d[:, :])
            a2t = cst.tile([128, 1], F32)
            nc.vector.memset(a2t[:], 0.2)

            icT = [wk.tile([128, BLK * 1024], BF16, name=f"ic{i}")
                   for i in range(2)]
            l2iT = [wk.tile([128, BLK, L2STR], BF16, name=f"l2i{i}")
                    for i in range(2)]
            l3iT = [wk.tile([128, 2, 10, 10, 2, BLK], BF16, name=f"l3i{i}")
                    for i in range(2)]
            l4iT = [wk.tile([128, 2, 6, 6, 2, GRP], BF16, name=f"l4i{i}")
                    for i in range(2)]
            for i in range(2):
                nc.vector.memset(icT[i][64:128, :], 0.0)
                nc.vector.memset(l2iT[i][:], 0.0)
                nc.vector.memset(l3iT[i][:], 0.0)
                nc.vector.memset(l4iT[i][:], 0.0)

            # persistent L1 psum: 4 banks x 2 col-halves = 8 quadrants.
            # L4's two accumulators alias banks 2..3 (temporally disjoint:
            # the L1 blocks that touch banks 2..3 run ~40 MM slots after
            # the L4 fo evictions of the previous group have drained).
            psL1 = psl1p.tile([128, 4, 512], F32, name="psl1")

            def emit_l1_group(b, g):
                """L1 for images 2g, 2g+1 of blk b (2 psum halves each)."""
                ic = icT[b % 2]
                l2i = l2iT[b % 2]
                for img in (2 * g, 2 * g + 1):
                    cg, bp = img % 2, (img % 4) // 2
                    for h in range(2):
                        psb = 2 * img + h
                        nc.tensor.matmul(psL1[64 * cg:64 * cg + 64,
                                              2 * bp + h, :],
                                         w1t[:, :],
                                         ic[:, psb * 512:(psb + 1) * 512],
                                         start=True, stop=True)
                    # one 1024-elem eviction per image across the bank pair
                    # (psum col order [r2, a, b, c] makes (h r2) merge)
                    src = psL1[64 * cg:64 * cg + 64,
                               2 * bp:2 * bp + 2, 0:512].rearrange(
                        "p h (r a b c) -> p (a b) (h r) c", r=8, a=2, b=2)
                    l2f = l2i[0:64, img, 0:L2IMG].rearrange(
                        "p (a b r c) -> p a b r c", a=2, b=2, r=18)
                    dst = l2f[:, :, :, 1:17, 1:17].rearrange(
                        "p a b r c -> p (a b) r c")
                    if zero_bias and img % 4 == 1:
                        # DVE 2-op LeakyReLU eviction (bias known zero)
                        tmp = stg.tile([64, 1024], F32, tag="l1tmp")
                        tv = tmp[:, :].rearrange(
                            "p (ab r c) -> p ab r c", ab=4, r=16)
                        nc.vector.tensor_scalar_mul(tv, src, 0.2)
                        nc.vector.tensor_tensor(dst, src, tv, op=MAX)
                    else:
                        nc.scalar.activation(dst, src, LRELU,
                                             bias=b1t[0:64, :],
                                             alpha=a2t[0:64, :])
                    # shifted upper copy: one flat SBUF->SBUF DMA (HWDGE)
                    nc.sync.dma_start(l2i[64:128, img, 0:L2IMG],
                                      l2i[0:64, img, 1:L2STR])

            def emit_l2_psb(b, psb):
                """L2 psum block psb (images 2psb, 2psb+1) of blk b."""
                l2i = l2iT[b % 2]
                l3i = l3iT[b % 2]
                j0 = 2 * psb
                mv = l2i[:, j0:j0 + 2, 0:L2IMG].rearrange(
                    "p i (a b r c) -> p i a b r c", a=2, b=2, r=18)
                ps = ps2.tile([128, 2, 16, 16], F32, tag="l2ps")
                # singles first as K=64 lower-half reads: they don't depend
                # on the shifted-copy DMA, covering its latency
                for s, (kh, kw) in enumerate(L2_SINGLES):
                    idx = 10 + s
                    nc.tensor.matmul(
                        ps[:], w2t[0:64, idx * 128:(idx + 1) * 128],
                        mv[0:64, :, kh % 2, 0, kh // 2:kh // 2 + 16, 2:18],
                        start=(s == 0), stop=False)
                for idx, (kh, kw) in enumerate(L2_PAIRS):
                    nc.tensor.matmul(
                        ps[:], w2t[:, idx * 128:(idx + 1) * 128],
                        mv[:, :, kh % 2, kw % 2, kh // 2:kh // 2 + 16,
                           kw // 2:kw // 2 + 16],
                        start=False, stop=(idx == 9))
                # evict into l3i (pc-inner layout): 2 merged 256-elem ops
                for pr in range(2):
                    src = ps[:, :, pr::2, :]
                    dst = l3i[:, pr, 1:9, 1:9, 0:2, j0:j0 + 2].rearrange(
                        "p r c pc i -> p i r (c pc)")
                    if zero_bias and pr == 1:
                        tmp = stg.tile([128, 256], F32, tag="l2tmp")
                        tv = tmp[:, 0:256].rearrange(
                            "p (i r c) -> p i r c", i=2, r=8)
                        nc.vector.tensor_scalar_mul(tv, src, 0.2)
                        nc.vector.tensor_tensor(dst, src, tv, op=MAX)
                    else:
                        nc.scalar.activation(dst, src, LRELU,
                                             bias=b2t[:, :], alpha=a2t[:, :])

            def emit_l3(b):
                """L3 for blk b: 2 ci planes x 25 taps, psum [r, c, img]."""
                l3i = l3iT[b % 2]
                sb4 = b % 4
                for cp in range(2):
                    ps3 = ps2.tile([128, 8, 8, BLK], F32, tag="l3ps")
                    for tap in range(25):
                        kh, kw = tap // 5, tap % 5
                        nc.tensor.matmul(
                            ps3[:],
                            w3t[:, (cp * 25 + tap) * 128:
                                (cp * 25 + tap + 1) * 128],
                            l3i[:, kh % 2, kh // 2:kh // 2 + 8,
                                kw // 2:kw // 2 + 8, kw % 2, :],
                            start=(tap == 0), stop=(tap == 24))
                    for pr in range(2):
                        nc.scalar.activation(
                            l4iT[cp][:, pr, 1:5, 1:5, 0:2,
                                     sb4 * BLK:(sb4 + 1) * BLK]
                            .rearrange("p r c pc i -> p r (c pc) i"),
                            ps3[:, pr::2, :, :],
                            LRELU, bias=b3t[:, cp:cp + 1], alpha=a2t[:, :])

            def emit_l4(grp):
                """L4 over a completed 32-image group: 4 sequential q-passes
                (one 50-MM chain each) so no fo eviction head-blocks the ACT
                queue for more than one chain."""
                for q in range(4):
                    p4 = psL1[:, 2 + q % 2, :].rearrange(
                        "p (r c i) -> p r c i", r=4, c=4)
                    for i4 in range(50):
                        cip, tap = i4 // 25, i4 % 25
                        kh, kw = tap // 5, tap % 5
                        w0 = (cip * 25 + tap) * 512 + q * 128
                        nc.tensor.matmul(
                            p4,
                            w4t[:, w0:w0 + 128],
                            l4iT[cip][:, kh % 2,
                                      kh // 2:kh // 2 + 4,
                                      kw // 2:kw // 2 + 4, kw % 2, :],
                            start=(i4 == 0), stop=(i4 == 49))
                    fo = stg.tile([128, GRP, 16], F32, tag="fo")
                    nc.scalar.activation(
                        fo[:], p4.rearrange("p r c i -> p i (r c)"),
                        LRELU, bias=b4t[:, q:q + 1], alpha=a2t[:, :])
                    nc.scalar.dma_start(
                        fd[:, q, grp * GRP:(grp + 1) * GRP, :],
                        fo[:])

            def dma_ic(b):
                c0 = b * BLK * 1024
                ic = icT[b % 2]
                nc.sync.dma_start(ic[0:38, :], icd[0:38, c0:c0 + BLK * 1024])
                nc.sync.dma_start(ic[38:75, :], icd[38:75, c0:c0 + BLK * 1024])

            # software pipeline: [L1(b) interleaved with L2(b-1)], L3(b-2), L4
            # ic first so L1(0) starts immediately; big weights follow
            # (w2/w3/w4 are first needed 1/2/5 iterations in)
            dma_ic(0)
            dma_ic(1)
            nc.scalar.dma_start(w2t[:], w2d[:, :])
            nc.scalar.dma_start(w3t[:], w3d[:, :])
            nc.scalar.dma_start(w4t[:], w4d[:, :])
            for b in range(nblk + 2):
                if b < nblk:
                    for g in range(4):
                        if b >= 1:
                            emit_l2_psb(b - 1, g)
                        emit_l1_group(b, g)
                    if b + 2 < nblk:
                        dma_ic(b + 2)
                elif b == nblk:
                    for g in range(4):
                        emit_l2_psb(b - 1, g)
                if 2 <= b <= nblk + 1:
                    emit_l3(b - 2)
                    if (b - 2) % 4 == 3:
                        emit_l4((b - 2) // 4)
    nc.compile()
    return nc


def _prep_inputs(x, W1, b1, W2, b2, W3, b3, W4, b4, nimg):
    """Host preprocessing -> per-core in_maps (shared weight arrays)."""
    import ml_dtypes
    bf16 = ml_dtypes.bfloat16
    f32 = np.float32
    n = x.shape[0]
    ncores = n // nimg
    xpad = np.pad(np.asarray(x, dtype=f32), ((0, 0), (0, 0), (2, 2), (2, 2)))
    s = xpad.strides
    v = np.lib.stride_tricks.as_strided(
        xpad, shape=(n, 3, 5, 5, 32, 32),
        strides=(s[0], s[1], s[2], s[3], 2 * s[2], 2 * s[3]))
    # column order per image: [h(2), r2(8), pr(2), pc(2), c2(16)]
    # row 32 = h*16 + r2*2 + pr ; col 32 = c2*2 + pc
    vr = v.reshape(n, 3, 5, 5, 2, 8, 2, 16, 2)      # rows->(h,r2,pr) cols->(c2,pc)
    vp = vr.transpose(1, 2, 3, 0, 4, 5, 6, 8, 7)    # [3,5,5,n,h,r2,pr,pc,c2]
    ic_all = np.ascontiguousarray(
        vp.reshape(75, n, 1024).astype(bf16))

    w1l = np.ascontiguousarray(
        np.asarray(W1, f32).transpose(1, 2, 3, 0).reshape(75, 64))
    w1h = np.zeros((128, 64), f32)
    w1h[0:75, 0:64] = w1l
    b1h = np.zeros((128, 1), f32)
    b1h[0:64, 0] = np.asarray(b1, f32)

    w2h = np.zeros((128, 15 * 128), f32)
    W2f = np.asarray(W2, f32)
    for idx, (kh, kw) in enumerate(L2_PAIRS):
        w2h[0:64, idx * 128:(idx + 1) * 128] = W2f[:, :, kh, kw].T
        w2h[64:128, idx * 128:(idx + 1) * 128] = W2f[:, :, kh, kw + 2].T
    for s_, (kh, kw) in enumerate(L2_SINGLES):
        idx = 10 + s_
        w2h[0:64, idx * 128:(idx + 1) * 128] = W2f[:, :, kh, kw].T
    b2h = np.asarray(b2, f32).reshape(128, 1)

    w3h = np.zeros((128, 2 * 25 * 128), f32)
    for cp in range(2):
        for t in range(25):
            kh, kw = t // 5, t % 5
            w3h[:, (cp * 25 + t) * 128:(cp * 25 + t + 1) * 128] = \
                np.asarray(W3, f32)[cp * 128:(cp + 1) * 128, :, kh, kw].T
    b3h = np.ascontiguousarray(
        np.asarray(b3, f32).reshape(2, 128).T)                   # [128,2]

    # w4 SBUF-resident layout: [(cip*25+tap)*512 + q*128 + m] columns
    w4h = np.zeros((128, 2 * 25 * 512), f32)
    for cip in range(2):
        for t in range(25):
            kh, kw = t // 5, t % 5
            w4h[:, (cip * 25 + t) * 512:(cip * 25 + t + 1) * 512] = \
                np.asarray(W4, f32)[:, cip * 128:(cip + 1) * 128, kh, kw].T
    b4h = np.ascontiguousarray(
        np.asarray(b4, f32).reshape(4, 128).T)                   # [128,4]

    w1h = w1h.astype(bf16)
    w2h = w2h.astype(bf16)
    w3h = w3h.astype(bf16)
    w4h = w4h.astype(bf16)

    in_maps = []
    for c in range(ncores):
        ic = np.ascontiguousarray(
            ic_all[:, c * nimg:(c + 1) * nimg, :].reshape(75, nimg * 1024))
        in_maps.append({"ic": ic, "w1": w1h, "w2": w2h, "w3": w3h,
                        "w4": w4h, "b1": b1h, "b2": b2h, "b3": b3h,
                        "b4": b4h})
    return in_maps


def _run(inputs, trace=False, nimg=512, ncores=NCORES):
    from concourse.bass_utils import run_bass_kernel_spmd

    zero_bias = not np.any(np.asarray(inputs["b1"]))
    key = (nimg, ncores, zero_bias)
    if key not in _CACHE:
        _CACHE[key] = _build_program(nimg, zero_bias)
    nc = _CACHE[key]

    in_maps = _prep_inputs(
        inputs["x"], inputs["W1"], inputs["b1"], inputs["W2"], inputs["b2"],
        inputs["W3"], inputs["b3"], inputs["W4"], inputs["b4"], nimg)

    res = run_bass_kernel_spmd(nc, in_maps, core_ids=list(range(ncores)),
                               trace=trace)
    feats = np.concatenate(
        [r["f"].transpose(2, 1, 0, 3).reshape(nimg, 8192)
         for r in res.results], axis=0)                          # [N, 8192]
    return feats, res


def kernel(**inputs):
    x = np.asarray(inputs["x"])
    n = x.shape[0]
    nimg = n // NCORES
    feats, _ = _run(inputs, trace=False, nimg=nimg)

    if int(np.asarray(inputs.get("is_local", 1))) == 0:
        return feats.astype(np.float32)

    batch_size = int(np.asarray(inputs["batch_size"]))
    seg = np.asarray(inputs["f_obj_to_img"]).astype(np.int64)
    nh = n // 2
    fake, real = feats[:nh], feats[nh:]
    counts = np.bincount(seg, minlength=batch_size).astype(np.float32)
    denom = np.maximum(counts, 1.0)[:, None]
    fsum = np.zeros((batch_size, 8192), np.float32)
    rsum = np.zeros((batch_size, 8192), np.float32)
    np.add.at(fsum, seg, fake)
    np.add.at(rsum, seg, real)
    favg = np.where((counts > 0)[:, None], fsum / denom, 0.0)
    ravg = np.where((counts > 0)[:, None], rsum / denom, 0.0)
    return np.concatenate([favg, ravg], axis=0).astype(np.float32)


# revision 48
# speedup vs baseline: 1.0148x; 1.0007x over previous
"""Trainium2 Bass kernel for nn_ContextualCritic (4-layer strided conv + segment mean).

Self-contained: kernel(**inputs) -> np.ndarray [2B, 8192].

Design (per core, data-parallel over 8 cores, 512 images each):
 - All matmul operands bf16, fp32 PSUM accumulation, bf16 evictions.
 - Software-pipelined stream per 8-image blk b: [L2(b-1) psb interleaved
   with L1(b) groups] -> L3(b-2) -> L4 (per 32-image group, 4 sequential
   q-passes), so every layer handoff is covered by independent PE work
   and the PE stays warm (full 2.4 GHz HAM state, ~216 ns per 512-col MM).
 - Parity-split activation layouts (stride-2 taps become dense plane reads).
 - L2 K=128 tap packing: l2i is img-major flat [128, 8, 1297]; lower 64
   partitions hold the 4 parity planes (18x18 halo), upper 64 hold a flat
   1-elem-shifted copy (one SBUF->SBUF HWDGE DMA per image; every wrapped
   element lands on a halo zero / pad).  Upper reads of tap (kh,kw) yield
   tap (kh,kw+2), so taps pair into K=128 matmuls: 5 K=64 lower-only
   singles first (cover the copy DMA latency), then 10 pairs = 15 MM
   slots per 2 images instead of 25 K=64 slots (176 slots/blk total).
 - L1: K=75, M=64, persistent 4-bank psum (8 quadrants via column-group
   tiling); psum col order [r2, pr, pc, c2] lets one 1024-elem 3D-AP
   eviction per image read both halves across a bank pair.  L4 aliases
   psum banks 2-3 (temporally disjoint).
 - Evictions balanced ACT/DVE (DVE 2-op lrelu where bias==0), all APs
   <=3 free dims after stride-merging; fo/fd on the scalar DMA queue so
   ic/copies on the sync queue are never head-blocked by L4 chains.
 - Segment mean on host from [N,8192] features (sorted segment ids).
"""
import numpy as np

BLK = 8        # images per block
GRP = 32       # images per L4 group (nimg = ngrp*GRP)
NCORES = 8
L2IMG = 2 * 2 * 18 * 18      # 1296 elems per image in l2i
L2STR = L2IMG + 1            # +1 pad elem so the flat shift stays in-tile

_CACHE = {}

# L2 tap pairing: pairs (kh, kw) + (kh, kw+2), singles (kh, 4)
L2_PAIRS = [(kh, kw) for kh in range(5) for kw in range(2)]
L2_SINGLES = [(kh, 4) for kh in range(5)]


def _build_program(nimg, zero_bias):
    from concourse import bacc, mybir
    import concourse.tile as tile

    BF16 = mybir.dt.bfloat16
    F32 = mybir.dt.float32
    LRELU = mybir.ActivationFunctionType.Prelu
    MAX = mybir.AluOpType.max

    nblk = nimg // BLK

    nc = bacc.Bacc(None, target_bir_lowering=False)

    icd = nc.dram_tensor("ic", [75, nimg * 1024], BF16, kind="ExternalInput")
    w1d = nc.dram_tensor("w1", [128, 64], BF16, kind="ExternalInput")
    w2d = nc.dram_tensor("w2", [128, 15 * 128], BF16, kind="ExternalInput")
    w3d = nc.dram_tensor("w3", [128, 2 * 25 * 128], BF16, kind="ExternalInput")
    w4d = nc.dram_tensor("w4", [128, 2 * 25 * 512], BF16, kind="ExternalInput")
    b1d = nc.dram_tensor("b1", [128, 1], F32, kind="ExternalInput")
    b2d = nc.dram_tensor("b2", [128, 1], F32, kind="ExternalInput")
    b3d = nc.dram_tensor("b3", [128, 2], F32, kind="ExternalInput")
    b4d = nc.dram_tensor("b4", [128, 4], F32, kind="ExternalInput")
    fd = nc.dram_tensor("f", [128, 4, nimg, 16], F32, kind="ExternalOutput")

    with tile.TileContext(nc) as tc:
        with tc.tile_pool(name="const", bufs=1) as cst, \
             tc.tile_pool(name="work", bufs=1) as wk, \
             tc.tile_pool(name="ps", bufs=2, space="PSUM") as ps2, \
             tc.tile_pool(name="psl1", bufs=1, space="PSUM") as psl1p, \
             tc.tile_pool(name="stg", bufs=2) as stg:
            w1t = cst.tile([128, 64], BF16)
            nc.sync.dma_start(w1t[:], w1d[:, :])
            w2t = cst.tile([128, 15 * 128], BF16)
            w3t = cst.tile([128, 2 * 25 * 128], BF16)
            w4t = cst.tile([128, 2 * 25 * 512], BF16)
            b1t = cst.tile([128, 1], F32)
            nc.sync.dma_start(b1t[:], b1d[:, :])
            b2t = cst.tile([128, 1], F32)
            nc.sync.dma_start(b2t[:], b2d[:, :])
            b3t = cst.tile([128, 2], F32)
            nc.sync.dma_start(b3t[:], b3d[:, :])
            b4t = cst.tile([128, 4], F32)
            nc.sync.dma_start(b4t[:], b4d[:, :])
            a2t = cst.tile([128, 1], F32)
            nc.vector.memset(a2t[:], 0.2)

            icT = [wk.tile([128, BLK * 1024], BF16, name=f"ic{i}")
                   for i in range(2)]
            l2iT = [wk.tile([128, BLK, L2STR], BF16, name=f"l2i{i}")
                    for i in range(2)]
            l3iT = [wk.tile([128, 2, 10, 10, 2, BLK], BF16, name=f"l3i{i}")
                    for i in range(2)]
            l4iT = [wk.tile([128, 2, 6, 6, 2, GRP], BF16, name=f"l4i{i}")
                    for i in range(2)]
            for i in range(2):
                nc.vector.memset(icT[i][64:128, :], 0.0)
                nc.vector.memset(l2iT[i][:], 0.0)
                nc.vector.memset(l3iT[i][:], 0.0)
                nc.vector.memset(l4iT[i][:], 0.0)

            # persistent L1 psum: 4 banks x 2 col-halves = 8 quadrants.
            # L4's two accumulators alias banks 2..3 (temporally disjoint:
            # the L1 blocks that touch banks 2..3 run ~40 MM slots after
            # the L4 fo evictions of the previous group have drained).
            psL1 = psl1p.tile([128, 4, 512], F32, name="psl1")

            def emit_l1_group(b, g):
                """L1 for images 2g, 2g+1 of blk b (2 psum halves each)."""
                ic = icT[b % 2]
                l2i = l2iT[b % 2]
                for img in (2 * g, 2 * g + 1):
                    cg, bp = img % 2, (img % 4) // 2
                    for h in range(2):
                        psb = 2 * img + h
                        nc.tensor.matmul(psL1[64 * cg:64 * cg + 64,
                                              2 * bp + h, :],
                                         w1t[:, :],
                                         ic[:, psb * 512:(psb + 1) * 512],
                                         start=True, stop=True)
                    # one 1024-elem eviction per image across the bank pair
                    # (psum col order [r2, a, b, c] makes (h r2) merge)
                    src = psL1[64 * cg:64 * cg + 64,
                               2 * bp:2 * bp + 2, 0:512].rearrange(
                        "p h (r a b c) -> p (a b) (h r) c", r=8, a=2, b=2)
                    l2f = l2i[0:64, img, 0:L2IMG].rearrange(
                        "p (a b r c) -> p a b r c", a=2, b=2, r=18)
                    dst = l2f[:, :, :, 1:17, 1:17].rearrange(
                        "p a b r c -> p (a b) r c")
                    if zero_bias and img % 4 == 1:
                        # DVE 2-op LeakyReLU eviction (bias known zero)
                        tmp = stg.tile([64, 1024], F32, tag="l1tmp")
                        tv = tmp[:, :].rearrange(
                            "p (ab r c) -> p ab r c", ab=4, r=16)
                        nc.vector.tensor_scalar_mul(tv, src, 0.2)
                        nc.vector.tensor_tensor(dst, src, tv, op=MAX)
                    else:
                        nc.scalar.activation(dst, src, LRELU,
                                             bias=b1t[0:64, :],
                                             alpha=a2t[0:64, :])
                    # shifted upper copy: one flat SBUF->SBUF DMA (HWDGE)
                    nc.sync.dma_start(l2i[64:128, img, 0:L2IMG],
                                      l2i[0:64, img, 1:L2STR])

            def emit_l2_psb(b, psb):
                """L2 psum block psb (images 2psb, 2psb+1) of blk b."""
                l2i = l2iT[b % 2]
                l3i = l3iT[b % 2]
                j0 = 2 * psb
                mv = l2i[:, j0:j0 + 2, 0:L2IMG].rearrange(
                    "p i (a b r c) -> p i a b r c", a=2, b=2, r=18)
                ps = ps2.tile([128, 2, 16, 16], F32, tag="l2ps")
                # singles first as K=64 lower-half reads: they don't depend
                # on the shifted-copy DMA, covering its latency
                for s, (kh, kw) in enumerate(L2_SINGLES):
                    idx = 10 + s
                    nc.tensor.matmul(
                        ps[:], w2t[0:64, idx * 128:(idx + 1) * 128],
                        mv[0:64, :, kh % 2, 0, kh // 2:kh // 2 + 16, 2:18],
                        start=(s == 0), stop=False)
                for idx, (kh, kw) in enumerate(L2_PAIRS):
                    nc.tensor.matmul(
                        ps[:], w2t[:, idx * 128:(idx + 1) * 128],
                        mv[:, :, kh % 2, kw % 2, kh // 2:kh // 2 + 16,
                           kw // 2:kw // 2 + 16],
                        start=False, stop=(idx == 9))
                # evict into l3i (pc-inner layout): 2 merged 256-elem ops
                for pr in range(2):
                    src = ps[:, :, pr::2, :]
                    dst = l3i[:, pr, 1:9, 1:9, 0:2, j0:j0 + 2].rearrange(
                        "p r c pc i -> p i r (c pc)")
                    if zero_bias and pr == 1:
                        tmp = stg.tile([128, 256], F32, tag="l2tmp")
                        tv = tmp[:, 0:256].rearrange(
                            "p (i r c) -> p i r c", i=2, r=8)
                        nc.vector.tensor_scalar_mul(tv, src, 0.2)
                        nc.vector.tensor_tensor(dst, src, tv, op=MAX)
                    else:
                        nc.scalar.activation(dst, src, LRELU,
                                             bias=b2t[:, :], alpha=a2t[:, :])

            def emit_l3(b):
                """L3 for blk b: 2 ci planes x 25 taps, psum [r, c, img]."""
                l3i = l3iT[b % 2]
                sb4 = b % 4
                for cp in range(2):
                    ps3 = ps2.tile([128, 8, 8, BLK], F32, tag="l3ps")
                    for tap in range(25):
                        kh, kw = tap // 5, tap % 5
                        nc.tensor.matmul(
                            ps3[:],
                            w3t[:, (cp * 25 + tap) * 128:
                                (cp * 25 + tap + 1) * 128],
                            l3i[:, kh % 2, kh // 2:kh // 2 + 8,
                                kw // 2:kw // 2 + 8, kw % 2, :],
                            start=(tap == 0), stop=(tap == 24))
                    for pr in range(2):
                        nc.scalar.activation(
                            l4iT[cp][:, pr, 1:5, 1:5, 0:2,
                                     sb4 * BLK:(sb4 + 1) * BLK]
                            .rearrange("p r c pc i -> p r (c pc) i"),
                            ps3[:, pr::2, :, :],
                            LRELU, bias=b3t[:, cp:cp + 1], alpha=a2t[:, :])

            def emit_l4(grp):
                """L4 over a completed 32-image group: 4 sequential q-passes
                (one 50-MM chain each) so no fo eviction head-blocks the ACT
                queue for more than one chain."""
                for q in range(4):
                    p4 = psL1[:, 2 + q % 2, :].rearrange(
                        "p (r c i) -> p r c i", r=4, c=4)
                    for i4 in range(50):
                        cip, tap = i4 // 25, i4 % 25
                        kh, kw = tap // 5, tap % 5
                        w0 = (cip * 25 + tap) * 512 + q * 128
                        nc.tensor.matmul(
                            p4,
                            w4t[:, w0:w0 + 128],
                            l4iT[cip][:, kh % 2,
                                      kh // 2:kh // 2 + 4,
                                      kw // 2:kw // 2 + 4, kw % 2, :],
                            start=(i4 == 0), stop=(i4 == 49))
                    fo = stg.tile([128, GRP, 16], F32, tag="fo")
                    nc.scalar.activation(
                        fo[:], p4.rearrange("p r c i -> p i (r c)"),
                        LRELU, bias=b4t[:, q:q + 1], alpha=a2t[:, :])
                    nc.scalar.dma_start(
                        fd[:, q, grp * GRP:(grp + 1) * GRP, :],
                        fo[:])

            def dma_ic(b):
                c0 = b * BLK * 1024
                ic = icT[b % 2]
                nc.sync.dma_start(ic[0:38, :], icd[0:38, c0:c0 + BLK * 1024])
                nc.sync.dma_start(ic[38:75, :], icd[38:75, c0:c0 + BLK * 1024])

            # software pipeline: [L1(b) interleaved with L2(b-1)], L3(b-2), L4
            # ic first so L1(0) starts immediately; big weights follow
            # (w2/w3/w4 are first needed 1/2/5 iterations in)
            dma_ic(0)
            dma_ic(1)
            nc.scalar.dma_start(w2t[:], w2d[:, :])
            nc.scalar.dma_start(w3t[:], w3d[:, :])
            nc.scalar.dma_start(w4t[:], w4d[:, :])
            for b in range(nblk + 2):
                if b < nblk:
                    for g in range(4):
                        if b >= 1:
                            emit_l2_psb(b - 1, g)
                        emit_l1_group(b, g)
                    if b + 2 < nblk:
                        dma_ic(b + 2)
                elif b == nblk:
                    for g in range(4):
                        emit_l2_psb(b - 1, g)
                if 2 <= b <= nblk + 1:
                    emit_l3(b - 2)
                    if (b - 2) % 4 == 3:
                        emit_l4((b - 2) // 4)
    nc.compile()
    return nc


def _prep_inputs(x, W1, b1, W2, b2, W3, b3, W4, b4, nimg):
    """Host preprocessing -> per-core in_maps (shared weight arrays)."""
    import ml_dtypes
    bf16 = ml_dtypes.bfloat16
    f32 = np.float32
    n = x.shape[0]
    ncores = n // nimg
    xpad = np.pad(np.asarray(x, dtype=f32), ((0, 0), (0, 0), (2, 2), (2, 2)))
    s = xpad.strides
    v = np.lib.stride_tricks.as_strided(
        xpad, shape=(n, 3, 5, 5, 32, 32),
        strides=(s[0], s[1], s[2], s[3], 2 * s[2], 2 * s[3]))
    # column order per image: [h(2), r2(8), pr(2), pc(2), c2(16)]
    # row 32 = h*16 + r2*2 + pr ; col 32 = c2*2 + pc
    vr = v.reshape(n, 3, 5, 5, 2, 8, 2, 16, 2)      # rows->(h,r2,pr) cols->(c2,pc)
    vp = vr.transpose(1, 2, 3, 0, 4, 5, 6, 8, 7)    # [3,5,5,n,h,r2,pr,pc,c2]
    ic_all = np.ascontiguousarray(
        vp.reshape(75, n, 1024).astype(bf16))

    w1l = np.ascontiguousarray(
        np.asarray(W1, f32).transpose(1, 2, 3, 0).reshape(75, 64))
    w1h = np.zeros((128, 64), f32)
    w1h[0:75, 0:64] = w1l
    b1h = np.zeros((128, 1), f32)
    b1h[0:64, 0] = np.asarray(b1, f32)

    w2h = np.zeros((128, 15 * 128), f32)
    W2f = np.asarray(W2, f32)
    for idx, (kh, kw) in enumerate(L2_PAIRS):
        w2h[0:64, idx * 128:(idx + 1) * 128] = W2f[:, :, kh, kw].T
        w2h[64:128, idx * 128:(idx + 1) * 128] = W2f[:, :, kh, kw + 2].T
    for s_, (kh, kw) in enumerate(L2_SINGLES):
        idx = 10 + s_
        w2h[0:64, idx * 128:(idx + 1) * 128] = W2f[:, :, kh, kw].T
    b2h = np.asarray(b2, f32).reshape(128, 1)

    w3h = np.zeros((128, 2 * 25 * 128), f32)
    for cp in range(2):
        for t in range(25):
            kh, kw = t // 5, t % 5
            w3h[:, (cp * 25 + t) * 128:(cp * 25 + t + 1) * 128] = \
                np.asarray(W3, f32)[cp * 128:(cp + 1) * 128, :, kh, kw].T
    b3h = np.ascontiguousarray(
        np.asarray(b3, f32).reshape(2, 128).T)                   # [128,2]

    # w4 SBUF-resident layout: [(cip*25+tap)*512 + q*128 + m] columns
    w4h = np.zeros((128, 2 * 25 * 512), f32)
    for cip in range(2):
        for t in range(25):
            kh, kw = t // 5, t % 5
            w4h[:, (cip * 25 + t) * 512:(cip * 25 + t + 1) * 512] = \
                np.asarray(W4, f32)[:, cip * 128:(cip + 1) * 128, kh, kw].T
    b4h = np.ascontiguousarray(
        np.asarray(b4, f32).reshape(4, 128).T)                   # [128,4]

    w1h = w1h.astype(bf16)
    w2h = w2h.astype(bf16)
    w3h = w3h.astype(bf16)
    w4h = w4h.astype(bf16)

    in_maps = []
    for c in range(ncores):
        ic = np.ascontiguousarray(
            ic_all[:, c * nimg:(c + 1) * nimg, :].reshape(75, nimg * 1024))
        in_maps.append({"ic": ic, "w1": w1h, "w2": w2h, "w3": w3h,
                        "w4": w4h, "b1": b1h, "b2": b2h, "b3": b3h,
                        "b4": b4h})
    return in_maps


def _run(inputs, trace=False, nimg=512, ncores=NCORES):
    from concourse.bass_utils import run_bass_kernel_spmd

    zero_bias = not np.any(np.asarray(inputs["b1"]))
    key = (nimg, ncores, zero_bias)
    if key not in _CACHE:
        _CACHE[key] = _build_program(nimg, zero_bias)
    nc = _CACHE[key]

    in_maps = _prep_inputs(
        inputs["x"], inputs["W1"], inputs["b1"], inputs["W2"], inputs["b2"],
        inputs["W3"], inputs["b3"], inputs["W4"], inputs["b4"], nimg)

    res = run_bass_kernel_spmd(nc, in_maps, core_ids=list(range(ncores)),
                               trace=trace)
    feats = np.concatenate(
        [r["f"].transpose(2, 1, 0, 3).reshape(nimg, 8192)
         for r in res.results], axis=0)                          # [N, 8192]
    return feats, res


def kernel(**inputs):
    x = np.asarray(inputs["x"])
    n = x.shape[0]
    nimg = n // NCORES
    feats, _ = _run(inputs, trace=False, nimg=nimg)

    if int(np.asarray(inputs.get("is_local", 1))) == 0:
        return feats.astype(np.float32)

    batch_size = int(np.asarray(inputs["batch_size"]))
    seg = np.asarray(inputs["f_obj_to_img"]).astype(np.int64)
    nh = n // 2
    fake, real = feats[:nh], feats[nh:]
    counts = np.bincount(seg, minlength=batch_size).astype(np.float32)
    denom = np.maximum(counts, 1.0)[:, None]
    fsum = np.zeros((batch_size, 8192), np.float32)
    rsum = np.zeros((batch_size, 8192), np.float32)
    np.add.at(fsum, seg, fake)
    np.add.at(rsum, seg, real)
    favg = np.where((counts > 0)[:, None], fsum / denom, 0.0)
    ravg = np.where((counts > 0)[:, None], rsum / denom, 0.0)
    return np.concatenate([favg, ravg], axis=0).astype(np.float32)
